# revision 24
# baseline (speedup 1.0000x reference)
"""Trainium2 Bass kernel for nn_CrossAttention (16-head cross attention).

Reference computation (fp32, s1=s2=2048, d1=d2=1024, H=16, DK=DV=64):
    q = x1 @ Wq.T ; k = x2 @ Wk.T ; v = x2 @ Wv.T      (per-head reshape)
    attn = softmax(q k^T / 8) per head
    out = LeakyReLU_0.01((attn v) @ Wo.T + bo)

Distribution (8 NeuronCores, tensor-parallel over heads):
  - Each core owns 2 heads: column-shards of Wq/Wk/Wv (128 rows each).
  - Inputs are fed pre-transposed from the host (x1.T, x2.T, W.T) so the
    contraction axis lands on SBUF partitions without any on-device
    transposition of the big activations.
  - Per-head attention computed in "transposed" orientation: S^T[j,i]
    tiles -> exp on ACT (no max subtraction needed: |scores|<~3 by
    construction) -> O'^T = [V|1]^T @ P^T which fuses the softmax
    denominator into the matmul (row 64 of the PSUM output = row sums).
    Scores matmuls slice q^T/k^T directly out of the projection buffers
    (K=64 at base partition 0/64 -> PE row-tiling, no pad or copies).
  - Normalized heads (bf16) are exchanged with AllToAlls (0.5 MB/core)
    so each core ends up with ALL heads for its 256-row slice of s1;
    the output projection then uses the full Wo (no reduction needed).
  - Epilogue (bias via K=1 ones-row matmul + leaky relu on DVE); output
    is the core's 256-row slice; the host concatenates the 8 slices.
"""

import numpy as np

import concourse.bass as bass
import concourse.mybir as mybir
import concourse.tile as tile
from concourse import bacc
from concourse import bass_utils
from concourse.masks import make_identity

NC_CORES = 8
S1 = 2048
S2 = 2048
D1 = 1024
D2 = 1024
H, DK, DV = 16, 64, 64
HPC = H // NC_CORES          # heads per core = 2
EPC = HPC * DK               # projection dims per core = 128
SPC = S1 // NC_CORES         # output rows per core = 256
P = 128
F32 = mybir.dt.float32
F32R = mybir.dt.float32r
ACT_EXP = mybir.ActivationFunctionType.Exp
ACT_LRELU = mybir.ActivationFunctionType.Lrelu
ACT_LN = mybir.ActivationFunctionType.Ln
MAX = mybir.AluOpType.max

NEG_SLOPE = 0.01
SCALE = 1.0 / np.sqrt(np.float32(DK))   # 0.125

S2_T = S2 // P               # 16 key tiles
KD1 = D1 // P                # 8 contraction tiles for projections
KDV = (H * DV) // P          # 8 contraction tiles for out projection


BF16 = mybir.dt.bfloat16
MM_DTYPE = "bf16"  # "bf16" | "f32r" | "f32" for matmul operand tiles


def build(mm_dtype: str = MM_DTYPE, single_core: bool = False):
    """single_core=True swaps the AllToAll for a local DMA copy (its exact
    1-core semantics) so the kernel can run in TimelineSim for perf
    estimation."""
    mmdt = {"bf16": BF16, "f32r": F32R, "f32": F32}[mm_dtype]
    nc = bacc.Bacc("TRN2", target_bir_lowering=False, debug=False,
                   num_devices=1 if single_core else NC_CORES)

    x1T = nc.dram_tensor("x1T", [D1, S1], mmdt, kind="ExternalInput")
    x2T = nc.dram_tensor("x2T", [D2, S2], mmdt, kind="ExternalInput")
    wqT = nc.dram_tensor("wqT", [D1, EPC], mmdt, kind="ExternalInput")
    wkT = nc.dram_tensor("wkT", [D2, EPC], mmdt, kind="ExternalInput")
    wvT = nc.dram_tensor("wvT", [D2, EPC], mmdt, kind="ExternalInput")
    woT = nc.dram_tensor("woT", [H * DV, D1], mmdt, kind="ExternalInput")
    bo_bc = nc.dram_tensor("bo_bc", [1, D1], F32, kind="ExternalInput")
    out = nc.dram_tensor("out", [SPC, D1], F32, kind="ExternalOutput")
    # exchange payload in the matmul dtype (bf16 halves the collective)
    import os as _osA
    a2a_dt = {"bf16": BF16, "f32": F32}[_osA.environ.get("A2A_DT", "bf16")] \
        if mmdt == BF16 else F32
    a2a_shared = _osA.environ.get("A2A_SHARED", "0") == "1"
    a2a_in = [nc.dram_tensor(f"a2a_in{h}", [NC_CORES * DV, SPC], a2a_dt,
                             kind="Internal") for h in range(HPC)]
    a2a_out = [nc.dram_tensor(f"a2a_out{h}", [NC_CORES * DV, SPC], a2a_dt,
                              kind="Internal",
                              addr_space="Shared" if a2a_shared else "Local")
              for h in range(HPC)]

    import os as _os0
    _ptb = int(_os0.environ.get("PTB", "5"))
    _xtb = int(_os0.environ.get("XTB", "10"))
    _psb = int(_os0.environ.get("PSB", "2"))
    _pob = int(_os0.environ.get("POB", "4"))
    _dgr = int(_os0.environ.get("DGR", "2"))

    with tile.TileContext(nc) as tc:
        with (
            tc.tile_pool(name="const", bufs=1) as cpool,
            tc.tile_pool(name="res", bufs=1) as rpool,
            tc.tile_pool(name="xin", bufs=_xtb) as xpool,
            tc.tile_pool(name="lhs", bufs=2) as lpool,
            tc.tile_pool(name="pt", bufs=_ptb) as ptpool,
            tc.tile_pool(name="ytmp", bufs=2) as ypool,
            tc.tile_pool(name="norm", bufs=2) as npool,
            tc.tile_pool(name="ps", bufs=_psb, space="PSUM") as pspool,
            tc.tile_pool(name="po", bufs=_pob, space="PSUM") as popool,
        ):
            # ---------------- constants (wo/bias deferred) ----------------
            ident = cpool.tile([P, P], F32 if mmdt == F32R else mmdt)
            make_identity(nc, ident[:])
            wq_sb = cpool.tile([P, KD1, EPC], mmdt)
            wk_sb = cpool.tile([P, KD1, EPC], mmdt)
            wv_sb = cpool.tile([P, KD1, EPC], mmdt)
            nc.sync.dma_start(wk_sb[:], wkT.rearrange("(o p) m -> p o m", p=P))
            nc.sync.dma_start(wv_sb[:], wvT.rearrange("(o p) m -> p o m", p=P))
            nc.sync.dma_start(wq_sb[:], wqT.rearrange("(o p) m -> p o m", p=P))
            wo_sb = cpool.tile([P, KDV, D1], mmdt)
            bo_r = cpool.tile([1, D1], F32R)
            ones_r = cpool.tile([1, P], F32R)
            nc.vector.memset(ones_r[:].bitcast(F32), 1.0)
            # all-ones row used as K=1 matmul lhsT to broadcast the softmax
            # denominator reciprocal across the DV output partitions
            ones_t = cpool.tile([1, DV], F32R)
            nc.vector.memset(ones_t[:].bitcast(F32), 1.0)

            # ---------------- residents ----------------
            vT = rpool.tile([P, S2], mmdt, name="vT")
            # per-head q^T/k^T, kept at their natural partition range
            # (h=0 -> rows 0:64, h=1 -> rows 64:128) with the other half
            # zeroed: partition-aligned DVE copies straight from the
            # projection PSUM, and the K=128 matmul sums the zeros away.
            qTh = [rpool.tile([P, S1], mmdt, name=f"qT{h}") for h in range(HPC)]
            kTh = [rpool.tile([P, S2], mmdt, name=f"kT{h}") for h in range(HPC)]
            # V natural + ones column, per key tile: [j, (v_h0|1|v_h1|1)]
            vP = rpool.tile([P, S2_T, 2 * (DV + 1)], mmdt)
            oTh = [rpool.tile([DV, S1], a2a_dt, name=f"oT{h}")
                   for h in range(HPC)]

            def msview(ap):
                return ap.bitcast(F32) if mmdt == F32R else ap

            for h in range(HPC):
                pad = slice(DK, P) if h == 0 else slice(0, DK)
                nc.vector.memset(msview(qTh[h][pad, :]), 0.0)
                nc.vector.memset(msview(kTh[h][pad, :]), 0.0)
            nc.vector.memset(msview(vP[:, :, DV:DV + 1]), 1.0)
            nc.vector.memset(msview(vP[:, :, 2 * DV + 1:2 * DV + 2]), 1.0)

            # ---------------- projections ----------------
            # K and V share one pass over x2T (each x2 tile DMA'd once).
            # x2 loads issue from gpsimd/SWDGE, x1 loads from SP/HWDGE to
            # spread DMA-issue cost across sequencers.
            x2v = x2T.rearrange("(o p) i -> p o i", p=P)
            x1v = x1T.rearrange("(o p) i -> p o i", p=P)

            def kv_proj(gp):
                gsl = slice(gp * 1024, (gp + 1) * 1024)
                pk = pspool.tile([P, 1024], F32, tag="ps", name=f"pk{gp}")
                pv = pspool.tile([P, 1024], F32, tag="ps", name=f"pv{gp}")
                for dg in range(KD1 // _dgr):
                    xt = xpool.tile([P, _dgr, 1024], mmdt, tag="xt",
                                    name=f"xt2_{gp}_{dg}")
                    # scalar/ACT ring is only free of exp work for gp0
                    eng = nc.gpsimd if (gp > 0 or dg % 2 == 0) else nc.scalar
                    eng.dma_start(
                        xt[:], x2v[:, _dgr * dg:_dgr * (dg + 1), gsl])
                    for dd in range(_dgr):
                        d = _dgr * dg + dd
                        for sg in range(2):
                            nc.tensor.matmul(
                                pk[:, sg * 512:(sg + 1) * 512],
                                wk_sb[:, d, :],
                                xt[:, dd, sg * 512:(sg + 1) * 512],
                                start=(d == 0), stop=(d == KD1 - 1))
                            nc.tensor.matmul(
                                pv[:, sg * 512:(sg + 1) * 512],
                                wv_sb[:, d, :],
                                xt[:, dd, sg * 512:(sg + 1) * 512],
                                start=(d == 0), stop=(d == KD1 - 1))
                nc.vector.tensor_copy(vT[:, gsl], pv[:])
                for h in range(HPC):
                    dat = slice(h * DK, (h + 1) * DK)
                    nc.vector.tensor_copy(kTh[h][dat, gsl], pk[dat, :])

            def q_proj(gp):
                gsl = slice(gp * 1024, (gp + 1) * 1024)
                pq = pspool.tile([P, 1024], F32, tag="ps", name=f"pq{gp}")
                for dg in range(KD1 // _dgr):
                    xt = xpool.tile([P, _dgr, 1024], mmdt, tag="xt",
                                    name=f"xt1_{gp}_{dg}")
                    nc.sync.dma_start(
                        xt[:], x1v[:, _dgr * dg:_dgr * (dg + 1), gsl])
                    for dd in range(_dgr):
                        d = _dgr * dg + dd
                        for sg in range(2):
                            nc.tensor.matmul(
                                pq[:, sg * 512:(sg + 1) * 512],
                                wq_sb[:, d, :],
                                xt[:, dd, sg * 512:(sg + 1) * 512],
                                start=(d == 0), stop=(d == KD1 - 1))
                for h in range(HPC):
                    dat = slice(h * DK, (h + 1) * DK)
                    nc.vector.tensor_copy(qTh[h][dat, gsl], pq[dat, :])

            def v_transpose(half):
                ptr = pspool.tile([P, 1024], mmdt if mmdt == BF16 else F32,
                                  tag="ps", name=f"ptr{half}")
                for k in range(8):
                    t = 8 * half + k
                    nc.tensor.transpose(
                        ptr[:, k * P:(k + 1) * P],
                        vT[:, t * P:(t + 1) * P].bitcast(F32)
                        if mmdt == F32R else vT[:, t * P:(t + 1) * P],
                        ident[:])
                for k in range(8):
                    t = 8 * half + k
                    nc.vector.tensor_copy(
                        vP[:, t, 0:DV], ptr[:, k * P:k * P + DV])
                    nc.vector.tensor_copy(
                        vP[:, t, DV + 1:2 * DV + 1],
                        ptr[:, k * P + DV:(k + 1) * P])

            # ---------------- attention ----------------
            po_tiles = {}

            def attn(h, ih, t0, t1):
                if (h, ih) not in po_tiles:
                    po_tiles[(h, ih)] = [
                        popool.tile([DV + 1, 512], F32, tag="po",
                                    name=f"po_{h}_{ih}_{gg}")
                        for gg in range(2)]
                po = po_tiles[(h, ih)]
                for t in range(t0, t1):
                    sps = pspool.tile([P, 1024], F32, tag="ps",
                                      name=f"sps_{h}_{ih}_{t}")
                    for sg in range(2):
                        i0 = ih * 1024 + sg * 512
                        nc.tensor.matmul(
                            sps[:, sg * 512:(sg + 1) * 512],
                            kTh[h][:, t * P:(t + 1) * P],
                            qTh[h][:, i0:i0 + 512],
                            start=True, stop=True)
                    ptt = ptpool.tile([P, 1024], mmdt, tag="ptt",
                                      name=f"ptt_{h}_{ih}_{t}")
                    nc.scalar.activation(ptt[:], sps[:], ACT_EXP,
                                         scale=float(SCALE))
                    for sg in range(2):
                        nc.tensor.matmul(
                            po[sg][:],
                            vP[:, t, h * (DV + 1):(h + 1) * (DV + 1)],
                            ptt[:, sg * 512:(sg + 1) * 512],
                            start=(t == 0), stop=(t == S2_T - 1))

            sr_tiles = {}

            def fin_recip(h, ih):
                # 1/Z = Exp(-Ln(Z)) on ACT (natural_log_exp_and_others
                # serves both: no table thrash, no 3.3us DVE reciprocal)
                po = po_tiles[(h, ih)]
                srs = []
                for gg in range(2):
                    g = ih * 2 + gg
                    sf = npool.tile([1, 512], F32, tag="sf",
                                    name=f"sf_{h}_{g}")
                    nc.vector.reciprocal(sf[:], po[gg][DV:DV + 1, :])
                    # the copy rounds to f32r (required by the bc matmul)
                    sr = npool.tile([1, 512], F32R, tag="sr",
                                    name=f"sr_{h}_{g}")
                    nc.vector.tensor_copy(sr[:], sf[:])
                    srs.append(sr)
                sr_tiles[(h, ih)] = srs

            def fin_apply(h, ih):
                # broadcast 1/Z over DV partitions (K=1 matmul), normalize
                # out of PSUM into the bf16 exchange buffer
                po = po_tiles.pop((h, ih))
                srs = sr_tiles.pop((h, ih))
                for gg in range(2):
                    g = ih * 2 + gg
                    gs = slice(g * 512, (g + 1) * 512)
                    bc = pspool.tile([DV, 512], F32, tag="ps",
                                     name=f"bc_{h}_{g}")
                    nc.tensor.matmul(
                        bc[:],
                        ones_t[:],
                        srs[gg][:],
                        start=True, stop=True)
                    nc.vector.tensor_copy(oTh[h][:, gs], po[gg][0:DV, :])
                    nc.vector.tensor_mul(
                        oTh[h][:, gs], oTh[h][:, gs], bc[:])

            def exchange(h):
                # scatter normalized head rows into this head's A2A buffer
                # then launch the exchange immediately so it overlaps the
                # remaining attention compute
                nc.sync.dma_start(
                    a2a_in[h].rearrange("(j p) i -> p j i", p=DV),
                    oTh[h][:].rearrange("p (j i) -> p j i", j=NC_CORES))
                if single_core:
                    nc.sync.dma_start(a2a_out[h][:], a2a_in[h][:])
                else:
                    nc.gpsimd.collective_compute(
                        "AllToAll", mybir.AluOpType.bypass,
                        replica_groups=[list(range(NC_CORES))],
                        ins=[a2a_in[h][:].opt()],
                        outs=[a2a_out[h][:].opt()],
                    )

            # out-proj lhsT split per exchange: ltA = [even heads | 0],
            # ltB = [0 | odd heads] so the even-head half of the projection
            # can run (K=128, full-rate) while the second AllToAll flies.
            ltA = [lpool.tile([P, KDV, P], mmdt, tag="ltA", name=f"ltA{it}")
                   for it in range(SPC // P)]
            ltB = [lpool.tile([P, KDV, P], mmdt, tag="ltB", name=f"ltB{it}")
                   for it in range(SPC // P)]
            for it in range(SPC // P):
                nc.vector.memset(msview(ltA[it][DV:P, :, :]), 0.0)
                nc.vector.memset(msview(ltB[it][0:DV, :, :]), 0.0)

            def lt_load(h):
                # prefetch this head's half of the out-proj lhsT row blocks
                lt = ltA if h == 0 else ltB
                for it in range(SPC // P):
                    nc.gpsimd.dma_start(
                        lt[it][h * DV:(h + 1) * DV, :, :],
                        a2a_out[h].rearrange("(k p) i -> p k i", p=DV)
                                  [:, :, it * P:(it + 1) * P])

            # ---------------- emission ----------------
            import os as _os
            _order = _os.environ.get("KERNEL_EMIT_ORDER", "ilv")

            def load_wo():
                nc.sync.dma_start(wo_sb[:],
                                  woT.rearrange("(o p) m -> p o m", p=P))
                nc.gpsimd.dma_start(bo_r[:], bo_bc[0:1, :])

            if _order == "ilv":
                kv_proj(0)
                q_proj(0)
                v_transpose(0)
                attn(0, 0, 0, 8)
                kv_proj(1)
                q_proj(1)
                v_transpose(1)
                attn(0, 0, 8, S2_T)
                load_wo()
                attn(0, 1, 0, 1)
                fin_recip(0, 0)
                attn(0, 1, 1, S2_T)
                fin_apply(0, 0)
                attn(1, 0, 0, 1)
                fin_recip(0, 1)
                attn(1, 0, 1, S2_T)
                fin_apply(0, 1)
                exchange(0)
                lt_load(0)
                attn(1, 1, 0, 1)
                fin_recip(1, 0)
                attn(1, 1, 1, S2_T)
                fin_apply(1, 0)
                fin_recip(1, 1)
                fin_apply(1, 1)
                exchange(1)
                lt_load(1)
            else:  # "seq"
                kv_proj(0)
                v_transpose(0)
                q_proj(0)
                kv_proj(1)
                v_transpose(1)
                q_proj(1)
                load_wo()
                for _h in range(HPC):
                    for _ih in range(2):
                        attn(_h, _ih, 0, S2_T)
                        fin_recip(_h, _ih)
                        fin_apply(_h, _ih)
                    exchange(_h)
                    lt_load(_h)

            # ---------------- output projection + epilogue ----------------
            # phase A (even heads, zero-padded K=128) overlaps the second
            # AllToAll; phase B adds the odd heads once it lands.
            pys = []
            for it in range(SPC // P):
                py = pspool.tile([P, D1], F32, tag="ps")
                pys.append(py)
                for k in range(KDV):
                    for ng in range(2):
                        nc.tensor.matmul(
                            py[:, ng * 512:(ng + 1) * 512],
                            ltA[it][:, k, :],
                            wo_sb[:, k, ng * 512:(ng + 1) * 512],
                            start=(k == 0), stop=False,
                            skip_group_check=True)
            for it in range(SPC // P):
                py = pys[it]
                for k in range(KDV):
                    for ng in range(2):
                        nc.tensor.matmul(
                            py[:, ng * 512:(ng + 1) * 512],
                            ltB[it][:, k, :],
                            wo_sb[:, k, ng * 512:(ng + 1) * 512],
                            start=False, stop=False,
                            skip_group_check=True)
                # bias folded into the same PSUM accumulation (K=1 ones row)
                for ng in range(2):
                    nc.tensor.matmul(
                        py[:, ng * 512:(ng + 1) * 512],
                        ones_r[:],
                        bo_r[:, ng * 512:(ng + 1) * 512],
                        start=False, stop=True, skip_group_check=True)
                ysb = ypool.tile([P, D1], F32, tag="ysb")
                yml = ypool.tile([P, D1], F32, tag="yml")
                nc.vector.tensor_scalar_mul(yml[:], py[:], NEG_SLOPE)
                nc.vector.tensor_tensor(ysb[:], py[:], yml[:], MAX)
                nc.sync.dma_start(out[it * P:(it + 1) * P, :], ysb[:])

    nc.compile()
    return nc


_NC_CACHE = {}


def _get_nc():
    if "nc" not in _NC_CACHE:
        _NC_CACHE["nc"] = build()
    return _NC_CACHE["nc"]


def make_in_maps(x1, x2, Wq, Wk, Wv, Wo, bo, mm_dtype: str = MM_DTYPE):
    import ml_dtypes
    cast = (lambda a: a.astype(ml_dtypes.bfloat16)) if mm_dtype == "bf16" \
        else (lambda a: a)
    x1 = np.asarray(x1, dtype=np.float32)
    x2 = np.asarray(x2, dtype=np.float32)
    Wq = np.asarray(Wq, dtype=np.float32)
    Wk = np.asarray(Wk, dtype=np.float32)
    Wv = np.asarray(Wv, dtype=np.float32)
    Wo = np.asarray(Wo, dtype=np.float32)
    bo = np.asarray(bo, dtype=np.float32)
    x1T = cast(np.ascontiguousarray(x1.T))
    x2T = cast(np.ascontiguousarray(x2.T))
    woT = cast(np.ascontiguousarray(Wo.T))
    bo_bc = np.ascontiguousarray(bo.reshape(1, D1))
    in_maps = []
    for c in range(NC_CORES):
        sl = slice(EPC * c, EPC * (c + 1))
        in_maps.append({
            "x1T": x1T,
            "x2T": x2T,
            "wqT": cast(np.ascontiguousarray(Wq[sl, :].T)),
            "wkT": cast(np.ascontiguousarray(Wk[sl, :].T)),
            "wvT": cast(np.ascontiguousarray(Wv[sl, :].T)),
            "woT": woT,
            "bo_bc": bo_bc,
        })
    return in_maps


def _install_profile_shim():
    """The image's antenv lacks axon_hooks; shim it so trace=True can pull
    NTFF profiles (exec_time_ns) through the axon tunnel."""
    import sys as _sys
    import types as _types
    try:
        from antenv.axon_hooks import get_axon_ntff_profile_hook  # noqa: F401
        return
    except ImportError:
        pass
    try:
        from trn_agent_boot.trn_boot import _ntff_profile_via_ctypes
        hook = _ntff_profile_via_ctypes("/opt/axon/libaxon_pjrt.so")
        mod = _types.ModuleType("antenv.axon_hooks")
        mod.get_axon_ntff_profile_hook = lambda: hook
        mod.set_axon_ntff_profile_hook = lambda h: None
        _sys.modules["antenv.axon_hooks"] = mod
        bass_utils.upload_artifacts = lambda tmpdir: tmpdir
    except Exception:
        pass


def run(inputs, trace=False):
    if trace:
        _install_profile_shim()
    nc = _get_nc()
    in_maps = make_in_maps(**inputs)
    res = bass_utils.run_bass_kernel_spmd(
        nc, in_maps, core_ids=list(range(NC_CORES)), trace=trace)
    full = np.concatenate(
        [res.results[c]["out"] for c in range(NC_CORES)], axis=0)
    return full, res


def kernel(**inputs):
    full, _ = run(inputs, trace=False)
    return full


# revision 25
# speedup vs baseline: 1.0669x; 1.0669x over previous
"""Trainium2 Bass kernel for nn_CrossAttention (16-head cross attention).

Reference computation (fp32, s1=s2=2048, d1=d2=1024, H=16, DK=DV=64):
    q = x1 @ Wq.T ; k = x2 @ Wk.T ; v = x2 @ Wv.T      (per-head reshape)
    attn = softmax(q k^T / 8) per head
    out = LeakyReLU_0.01((attn v) @ Wo.T + bo)

Distribution (8 NeuronCores, tensor-parallel over heads):
  - Each core owns 2 heads: column-shards of Wq/Wk/Wv (128 rows each).
  - Inputs are fed pre-transposed from the host (x1.T, x2.T, W.T) so the
    contraction axis lands on SBUF partitions without any on-device
    transposition of the big activations.
  - Per-head attention computed in "transposed" orientation: S^T[j,i]
    tiles -> exp on ACT (no max subtraction needed: |scores|<~3 by
    construction) -> O'^T = [V|1]^T @ P^T which fuses the softmax
    denominator into the matmul (row 64 of the PSUM output = row sums).
    Scores matmuls slice q^T/k^T directly out of the projection buffers
    (K=64 at base partition 0/64 -> PE row-tiling, no pad or copies).
  - Normalized heads (bf16) are exchanged with AllToAlls (0.5 MB/core)
    so each core ends up with ALL heads for its 256-row slice of s1;
    the output projection then uses the full Wo (no reduction needed).
  - Epilogue (bias via K=1 ones-row matmul + leaky relu on DVE); output
    is the core's 256-row slice; the host concatenates the 8 slices.
"""

import numpy as np

import concourse.bass as bass
import concourse.mybir as mybir
import concourse.tile as tile
from concourse import bacc
from concourse import bass_utils
from concourse.masks import make_identity

NC_CORES = 8
S1 = 2048
S2 = 2048
D1 = 1024
D2 = 1024
H, DK, DV = 16, 64, 64
HPC = H // NC_CORES          # heads per core = 2
EPC = HPC * DK               # projection dims per core = 128
SPC = S1 // NC_CORES         # output rows per core = 256
P = 128
F32 = mybir.dt.float32
F32R = mybir.dt.float32r
ACT_EXP = mybir.ActivationFunctionType.Exp
ACT_LRELU = mybir.ActivationFunctionType.Lrelu
ACT_LN = mybir.ActivationFunctionType.Ln
MAX = mybir.AluOpType.max

NEG_SLOPE = 0.01
SCALE = 1.0 / np.sqrt(np.float32(DK))   # 0.125

S2_T = S2 // P               # 16 key tiles
KD1 = D1 // P                # 8 contraction tiles for projections
KDV = (H * DV) // P          # 8 contraction tiles for out projection


BF16 = mybir.dt.bfloat16
MM_DTYPE = "bf16"  # "bf16" | "f32r" | "f32" for matmul operand tiles


def build(mm_dtype: str = MM_DTYPE, single_core: bool = False):
    """single_core=True swaps the AllToAll for a local DMA copy (its exact
    1-core semantics) so the kernel can run in TimelineSim for perf
    estimation."""
    mmdt = {"bf16": BF16, "f32r": F32R, "f32": F32}[mm_dtype]
    nc = bacc.Bacc("TRN2", target_bir_lowering=False, debug=False,
                   num_devices=1 if single_core else NC_CORES)

    x1T = nc.dram_tensor("x1T", [D1, S1], mmdt, kind="ExternalInput")
    x2T = nc.dram_tensor("x2T", [D2, S2], mmdt, kind="ExternalInput")
    wqT = nc.dram_tensor("wqT", [D1, EPC], mmdt, kind="ExternalInput")
    wkT = nc.dram_tensor("wkT", [D2, EPC], mmdt, kind="ExternalInput")
    wvT = nc.dram_tensor("wvT", [D2, EPC], mmdt, kind="ExternalInput")
    woT = nc.dram_tensor("woT", [H * DV, D1], mmdt, kind="ExternalInput")
    bo_bc = nc.dram_tensor("bo_bc", [1, D1], F32, kind="ExternalInput")
    out = nc.dram_tensor("out", [SPC, D1], F32, kind="ExternalOutput")
    # exchange payload in the matmul dtype (bf16 halves the collective)
    import os as _osA
    a2a_dt = {"bf16": BF16, "f32": F32}[_osA.environ.get("A2A_DT", "bf16")] \
        if mmdt == BF16 else F32
    a2a_shared = _osA.environ.get("A2A_SHARED", "0") == "1"
    a2a_in = [nc.dram_tensor(f"a2a_in{h}", [NC_CORES * DV, SPC], a2a_dt,
                             kind="Internal") for h in range(HPC)]
    a2a_out = [nc.dram_tensor(f"a2a_out{h}", [NC_CORES * DV, SPC], a2a_dt,
                              kind="Internal",
                              addr_space="Shared" if a2a_shared else "Local")
              for h in range(HPC)]

    import os as _os0
    _ptb = int(_os0.environ.get("PTB", "5"))
    _xtb = int(_os0.environ.get("XTB", "10"))
    _psb = int(_os0.environ.get("PSB", "2"))
    _pob = int(_os0.environ.get("POB", "4"))
    _dgr = int(_os0.environ.get("DGR", "2"))

    with tile.TileContext(nc) as tc:
        with (
            tc.tile_pool(name="const", bufs=1) as cpool,
            tc.tile_pool(name="res", bufs=1) as rpool,
            tc.tile_pool(name="xin", bufs=_xtb) as xpool,
            tc.tile_pool(name="lhs", bufs=2) as lpool,
            tc.tile_pool(name="pt", bufs=_ptb) as ptpool,
            tc.tile_pool(name="ytmp", bufs=2) as ypool,
            tc.tile_pool(name="norm", bufs=2) as npool,
            tc.tile_pool(name="ps", bufs=_psb, space="PSUM") as pspool,
            tc.tile_pool(name="po", bufs=_pob, space="PSUM") as popool,
        ):
            # ---------------- constants (wo/bias deferred) ----------------
            ident = cpool.tile([P, P], F32 if mmdt == F32R else mmdt)
            make_identity(nc, ident[:])
            wq_sb = cpool.tile([P, KD1, EPC], mmdt)
            wk_sb = cpool.tile([P, KD1, EPC], mmdt)
            wv_sb = cpool.tile([P, KD1, EPC], mmdt)
            nc.sync.dma_start(wk_sb[:], wkT.rearrange("(o p) m -> p o m", p=P))
            nc.sync.dma_start(wv_sb[:], wvT.rearrange("(o p) m -> p o m", p=P))
            nc.sync.dma_start(wq_sb[:], wqT.rearrange("(o p) m -> p o m", p=P))
            wo_sb = cpool.tile([P, KDV, D1], mmdt)
            bo_r = cpool.tile([1, D1], F32R)
            ones_r = cpool.tile([1, P], F32R)
            nc.vector.memset(ones_r[:].bitcast(F32), 1.0)
            # all-ones row used as K=1 matmul lhsT to broadcast the softmax
            # denominator reciprocal across the DV output partitions
            ones_t = cpool.tile([1, DV], F32R)
            nc.vector.memset(ones_t[:].bitcast(F32), 1.0)

            # ---------------- residents ----------------
            vT = rpool.tile([P, S2], mmdt, name="vT")
            # per-head q^T/k^T, kept at their natural partition range
            # (h=0 -> rows 0:64, h=1 -> rows 64:128) with the other half
            # zeroed: partition-aligned DVE copies straight from the
            # projection PSUM, and the K=128 matmul sums the zeros away.
            qTh = [rpool.tile([P, S1], mmdt, name=f"qT{h}") for h in range(HPC)]
            kTh = [rpool.tile([P, S2], mmdt, name=f"kT{h}") for h in range(HPC)]
            # V natural + ones column, per key tile: [j, (v_h0|1|v_h1|1)]
            vP = rpool.tile([P, S2_T, 2 * (DV + 1)], mmdt)
            oTh = [rpool.tile([DV, S1], a2a_dt, name=f"oT{h}")
                   for h in range(HPC)]

            def msview(ap):
                return ap.bitcast(F32) if mmdt == F32R else ap

            for h in range(HPC):
                pad = slice(DK, P) if h == 0 else slice(0, DK)
                nc.vector.memset(msview(qTh[h][pad, :]), 0.0)
                nc.vector.memset(msview(kTh[h][pad, :]), 0.0)
            nc.vector.memset(msview(vP[:, :, DV:DV + 1]), 1.0)
            nc.vector.memset(msview(vP[:, :, 2 * DV + 1:2 * DV + 2]), 1.0)

            # ---------------- projections ----------------
            # K and V share one pass over x2T (each x2 tile DMA'd once).
            # x2 loads issue from gpsimd/SWDGE, x1 loads from SP/HWDGE to
            # spread DMA-issue cost across sequencers.
            x2v = x2T.rearrange("(o p) i -> p o i", p=P)
            x1v = x1T.rearrange("(o p) i -> p o i", p=P)

            def kv_proj(gp):
                gsl = slice(gp * 1024, (gp + 1) * 1024)
                pk = pspool.tile([P, 1024], F32, tag="ps", name=f"pk{gp}")
                pv = pspool.tile([P, 1024], F32, tag="ps", name=f"pv{gp}")
                for dg in range(KD1 // _dgr):
                    xt = xpool.tile([P, _dgr, 1024], mmdt, tag="xt",
                                    name=f"xt2_{gp}_{dg}")
                    # scalar/ACT ring is only free of exp work for gp0
                    eng = nc.gpsimd if (gp > 0 or dg % 2 == 0) else nc.scalar
                    eng.dma_start(
                        xt[:], x2v[:, _dgr * dg:_dgr * (dg + 1), gsl])
                    for dd in range(_dgr):
                        d = _dgr * dg + dd
                        for sg in range(2):
                            nc.tensor.matmul(
                                pk[:, sg * 512:(sg + 1) * 512],
                                wk_sb[:, d, :],
                                xt[:, dd, sg * 512:(sg + 1) * 512],
                                start=(d == 0), stop=(d == KD1 - 1))
                            nc.tensor.matmul(
                                pv[:, sg * 512:(sg + 1) * 512],
                                wv_sb[:, d, :],
                                xt[:, dd, sg * 512:(sg + 1) * 512],
                                start=(d == 0), stop=(d == KD1 - 1))
                nc.vector.tensor_copy(vT[:, gsl], pv[:])
                for h in range(HPC):
                    dat = slice(h * DK, (h + 1) * DK)
                    nc.vector.tensor_copy(kTh[h][dat, gsl], pk[dat, :])

            def q_proj(gp):
                gsl = slice(gp * 1024, (gp + 1) * 1024)
                pq = pspool.tile([P, 1024], F32, tag="ps", name=f"pq{gp}")
                for dg in range(KD1 // _dgr):
                    xt = xpool.tile([P, _dgr, 1024], mmdt, tag="xt",
                                    name=f"xt1_{gp}_{dg}")
                    nc.sync.dma_start(
                        xt[:], x1v[:, _dgr * dg:_dgr * (dg + 1), gsl])
                    for dd in range(_dgr):
                        d = _dgr * dg + dd
                        for sg in range(2):
                            nc.tensor.matmul(
                                pq[:, sg * 512:(sg + 1) * 512],
                                wq_sb[:, d, :],
                                xt[:, dd, sg * 512:(sg + 1) * 512],
                                start=(d == 0), stop=(d == KD1 - 1))
                for h in range(HPC):
                    dat = slice(h * DK, (h + 1) * DK)
                    nc.vector.tensor_copy(qTh[h][dat, gsl], pq[dat, :])

            def v_transpose(half):
                ptr = pspool.tile([P, 1024], mmdt if mmdt == BF16 else F32,
                                  tag="ps", name=f"ptr{half}")
                for k in range(8):
                    t = 8 * half + k
                    nc.tensor.transpose(
                        ptr[:, k * P:(k + 1) * P],
                        vT[:, t * P:(t + 1) * P].bitcast(F32)
                        if mmdt == F32R else vT[:, t * P:(t + 1) * P],
                        ident[:])
                for k in range(8):
                    t = 8 * half + k
                    nc.vector.tensor_copy(
                        vP[:, t, 0:DV], ptr[:, k * P:k * P + DV])
                    nc.vector.tensor_copy(
                        vP[:, t, DV + 1:2 * DV + 1],
                        ptr[:, k * P + DV:(k + 1) * P])

            # ---------------- attention ----------------
            po_tiles = {}

            def attn(h, ih, t0, t1):
                if (h, ih) not in po_tiles:
                    po_tiles[(h, ih)] = [
                        popool.tile([DV + 1, 512], F32, tag="po",
                                    name=f"po_{h}_{ih}_{gg}")
                        for gg in range(2)]
                po = po_tiles[(h, ih)]
                for t in range(t0, t1):
                    sps = pspool.tile([P, 1024], F32, tag="ps",
                                      name=f"sps_{h}_{ih}_{t}")
                    for sg in range(2):
                        i0 = ih * 1024 + sg * 512
                        nc.tensor.matmul(
                            sps[:, sg * 512:(sg + 1) * 512],
                            kTh[h][:, t * P:(t + 1) * P],
                            qTh[h][:, i0:i0 + 512],
                            start=True, stop=True)
                    ptt = ptpool.tile([P, 1024], mmdt, tag="ptt",
                                      name=f"ptt_{h}_{ih}_{t}")
                    nc.scalar.activation(ptt[:], sps[:], ACT_EXP,
                                         scale=float(SCALE))
                    for sg in range(2):
                        nc.tensor.matmul(
                            po[sg][:],
                            vP[:, t, h * (DV + 1):(h + 1) * (DV + 1)],
                            ptt[:, sg * 512:(sg + 1) * 512],
                            start=(t == 0), stop=(t == S2_T - 1))

            sr_tiles = {}

            def fin_recip(h, ih, use_act=False):
                # deferred reciprocal of the denominator row; DVE while the
                # exp table is still in use, ACT 1/Z = Exp(-Ln(Z)) for the
                # final one (all score exps done -> one table switch)
                po = po_tiles[(h, ih)]
                srs = []
                for gg in range(2):
                    g = ih * 2 + gg
                    sf = npool.tile([1, 512], F32, tag="sf",
                                    name=f"sf_{h}_{g}")
                    sr = npool.tile([1, 512], F32R, tag="sr",
                                    name=f"sr_{h}_{g}")
                    if use_act:
                        nc.scalar.activation(sf[:], po[gg][DV:DV + 1, :],
                                             ACT_LN)
                        nc.scalar.activation(sr[:], sf[:], ACT_EXP,
                                             scale=-1.0)
                    else:
                        nc.vector.reciprocal(sf[:], po[gg][DV:DV + 1, :])
                        # the copy rounds to f32r (for the bc matmul)
                        nc.vector.tensor_copy(sr[:], sf[:])
                    srs.append(sr)
                sr_tiles[(h, ih)] = srs

            def fin_apply(h, ih):
                # broadcast 1/Z over DV partitions (K=1 matmul), normalize
                # out of PSUM into the bf16 exchange buffer
                po = po_tiles.pop((h, ih))
                srs = sr_tiles.pop((h, ih))
                for gg in range(2):
                    g = ih * 2 + gg
                    gs = slice(g * 512, (g + 1) * 512)
                    bc = pspool.tile([DV, 512], F32, tag="ps",
                                     name=f"bc_{h}_{g}")
                    nc.tensor.matmul(
                        bc[:],
                        ones_t[:],
                        srs[gg][:],
                        start=True, stop=True)
                    nc.vector.tensor_copy(oTh[h][:, gs], po[gg][0:DV, :])
                    nc.vector.tensor_mul(
                        oTh[h][:, gs], oTh[h][:, gs], bc[:])

            def exchange(h):
                # scatter normalized head rows into this head's A2A buffer
                # then launch the exchange immediately so it overlaps the
                # remaining attention compute
                nc.sync.dma_start(
                    a2a_in[h].rearrange("(j p) i -> p j i", p=DV),
                    oTh[h][:].rearrange("p (j i) -> p j i", j=NC_CORES))
                if single_core:
                    nc.sync.dma_start(a2a_out[h][:], a2a_in[h][:])
                else:
                    nc.gpsimd.collective_compute(
                        "AllToAll", mybir.AluOpType.bypass,
                        replica_groups=[list(range(NC_CORES))],
                        ins=[a2a_in[h][:].opt()],
                        outs=[a2a_out[h][:].opt()],
                    )

            # out-proj lhsT split per exchange: ltA = [even heads | 0],
            # ltB = [0 | odd heads] so the even-head half of the projection
            # can run (K=128, full-rate) while the second AllToAll flies.
            ltA = [lpool.tile([P, KDV, P], mmdt, tag="ltA", name=f"ltA{it}")
                   for it in range(SPC // P)]
            ltB = [lpool.tile([P, KDV, P], mmdt, tag="ltB", name=f"ltB{it}")
                   for it in range(SPC // P)]
            for it in range(SPC // P):
                nc.vector.memset(msview(ltA[it][DV:P, :, :]), 0.0)
                nc.vector.memset(msview(ltB[it][0:DV, :, :]), 0.0)

            def lt_load(h):
                # prefetch this head's half of the out-proj lhsT row blocks
                lt = ltA if h == 0 else ltB
                for it in range(SPC // P):
                    nc.gpsimd.dma_start(
                        lt[it][h * DV:(h + 1) * DV, :, :],
                        a2a_out[h].rearrange("(k p) i -> p k i", p=DV)
                                  [:, :, it * P:(it + 1) * P])

            pys = []

            def outproj_phaseA():
                # even heads (zero-padded to K=128): runs while the second
                # AllToAll is in flight, keeps the PE p-state warm
                for it in range(SPC // P):
                    py = pspool.tile([P, D1], F32, tag="ps")
                    pys.append(py)
                    for k in range(KDV):
                        for ng in range(2):
                            nc.tensor.matmul(
                                py[:, ng * 512:(ng + 1) * 512],
                                ltA[it][:, k, :],
                                wo_sb[:, k, ng * 512:(ng + 1) * 512],
                                start=(k == 0), stop=False,
                                skip_group_check=True)

            # ---------------- emission ----------------
            import os as _os
            _order = _os.environ.get("KERNEL_EMIT_ORDER", "ilv")

            def load_wo():
                nc.sync.dma_start(wo_sb[:],
                                  woT.rearrange("(o p) m -> p o m", p=P))
                nc.gpsimd.dma_start(bo_r[:], bo_bc[0:1, :])

            if _order == "ilv":
                kv_proj(0)
                q_proj(0)
                v_transpose(0)
                attn(0, 0, 0, 8)
                kv_proj(1)
                q_proj(1)
                v_transpose(1)
                attn(0, 0, 8, S2_T)
                load_wo()
                attn(0, 1, 0, 1)
                fin_recip(0, 0)
                attn(0, 1, 1, S2_T)
                fin_apply(0, 0)
                attn(1, 0, 0, 1)
                fin_recip(0, 1)
                attn(1, 0, 1, S2_T)
                fin_apply(0, 1)
                exchange(0)
                lt_load(0)
                attn(1, 1, 0, 1)
                fin_recip(1, 0)
                attn(1, 1, 1, S2_T)
                fin_apply(1, 0)
                fin_recip(1, 1, use_act=True)
                fin_apply(1, 1)
                exchange(1)
                outproj_phaseA()
                lt_load(1)
            else:  # "seq"
                kv_proj(0)
                v_transpose(0)
                q_proj(0)
                kv_proj(1)
                v_transpose(1)
                q_proj(1)
                load_wo()
                for _h in range(HPC):
                    for _ih in range(2):
                        attn(_h, _ih, 0, S2_T)
                        fin_recip(_h, _ih)
                        fin_apply(_h, _ih)
                    if _h == HPC - 1:
                        exchange(_h)
                        outproj_phaseA()
                    else:
                        exchange(_h)
                    lt_load(_h)

            # ---------------- output projection + epilogue ----------------
            # (phase A was emitted before the second AllToAll launch)
            for it in range(SPC // P):
                py = pys[it]
                for k in range(KDV):
                    for ng in range(2):
                        nc.tensor.matmul(
                            py[:, ng * 512:(ng + 1) * 512],
                            ltB[it][:, k, :],
                            wo_sb[:, k, ng * 512:(ng + 1) * 512],
                            start=False, stop=False,
                            skip_group_check=True)
                # bias folded into the same PSUM accumulation (K=1 ones row)
                for ng in range(2):
                    nc.tensor.matmul(
                        py[:, ng * 512:(ng + 1) * 512],
                        ones_r[:],
                        bo_r[:, ng * 512:(ng + 1) * 512],
                        start=False, stop=True, skip_group_check=True)
                ysb = ypool.tile([P, D1], F32, tag="ysb")
                yml = ypool.tile([P, D1], F32, tag="yml")
                nc.vector.tensor_scalar_mul(yml[:], py[:], NEG_SLOPE)
                nc.vector.tensor_tensor(ysb[:], py[:], yml[:], MAX)
                nc.sync.dma_start(out[it * P:(it + 1) * P, :], ysb[:])

    nc.compile()
    return nc


_NC_CACHE = {}


def _get_nc():
    if "nc" not in _NC_CACHE:
        _NC_CACHE["nc"] = build()
    return _NC_CACHE["nc"]


def make_in_maps(x1, x2, Wq, Wk, Wv, Wo, bo, mm_dtype: str = MM_DTYPE):
    import ml_dtypes
    cast = (lambda a: a.astype(ml_dtypes.bfloat16)) if mm_dtype == "bf16" \
        else (lambda a: a)
    x1 = np.asarray(x1, dtype=np.float32)
    x2 = np.asarray(x2, dtype=np.float32)
    Wq = np.asarray(Wq, dtype=np.float32)
    Wk = np.asarray(Wk, dtype=np.float32)
    Wv = np.asarray(Wv, dtype=np.float32)
    Wo = np.asarray(Wo, dtype=np.float32)
    bo = np.asarray(bo, dtype=np.float32)
    x1T = cast(np.ascontiguousarray(x1.T))
    x2T = cast(np.ascontiguousarray(x2.T))
    woT = cast(np.ascontiguousarray(Wo.T))
    bo_bc = np.ascontiguousarray(bo.reshape(1, D1))
    in_maps = []
    for c in range(NC_CORES):
        sl = slice(EPC * c, EPC * (c + 1))
        in_maps.append({
            "x1T": x1T,
            "x2T": x2T,
            "wqT": cast(np.ascontiguousarray(Wq[sl, :].T)),
            "wkT": cast(np.ascontiguousarray(Wk[sl, :].T)),
            "wvT": cast(np.ascontiguousarray(Wv[sl, :].T)),
            "woT": woT,
            "bo_bc": bo_bc,
        })
    return in_maps


def _install_profile_shim():
    """The image's antenv lacks axon_hooks; shim it so trace=True can pull
    NTFF profiles (exec_time_ns) through the axon tunnel."""
    import sys as _sys
    import types as _types
    try:
        from antenv.axon_hooks import get_axon_ntff_profile_hook  # noqa: F401
        return
    except ImportError:
        pass
    try:
        from trn_agent_boot.trn_boot import _ntff_profile_via_ctypes
        hook = _ntff_profile_via_ctypes("/opt/axon/libaxon_pjrt.so")
        mod = _types.ModuleType("antenv.axon_hooks")
        mod.get_axon_ntff_profile_hook = lambda: hook
        mod.set_axon_ntff_profile_hook = lambda h: None
        _sys.modules["antenv.axon_hooks"] = mod
        bass_utils.upload_artifacts = lambda tmpdir: tmpdir
    except Exception:
        pass


def run(inputs, trace=False):
    if trace:
        _install_profile_shim()
    nc = _get_nc()
    in_maps = make_in_maps(**inputs)
    res = bass_utils.run_bass_kernel_spmd(
        nc, in_maps, core_ids=list(range(NC_CORES)), trace=trace)
    full = np.concatenate(
        [res.results[c]["out"] for c in range(NC_CORES)], axis=0)
    return full, res


def kernel(**inputs):
    full, _ = run(inputs, trace=False)
    return full


# revision 26
# speedup vs baseline: 1.1327x; 1.0616x over previous
"""Trainium2 Bass kernel for nn_CrossAttention (16-head cross attention).

Reference computation (fp32, s1=s2=2048, d1=d2=1024, H=16, DK=DV=64):
    q = x1 @ Wq.T ; k = x2 @ Wk.T ; v = x2 @ Wv.T      (per-head reshape)
    attn = softmax(q k^T / 8) per head
    out = LeakyReLU_0.01((attn v) @ Wo.T + bo)

Distribution (8 NeuronCores, tensor-parallel over heads):
  - Each core owns 2 heads: column-shards of Wq/Wk/Wv (128 rows each).
  - Inputs are fed pre-transposed from the host (x1.T, x2.T, W.T) so the
    contraction axis lands on SBUF partitions without any on-device
    transposition of the big activations.
  - Per-head attention computed in "transposed" orientation: S^T[j,i]
    tiles -> exp on ACT (no max subtraction needed: |scores|<~3 by
    construction) -> O'^T = [V|1]^T @ P^T which fuses the softmax
    denominator into the matmul (row 64 of the PSUM output = row sums).
    Scores matmuls slice q^T/k^T directly out of the projection buffers
    (K=64 at base partition 0/64 -> PE row-tiling, no pad or copies).
  - Normalized heads (bf16) are exchanged with AllToAlls (0.5 MB/core)
    so each core ends up with ALL heads for its 256-row slice of s1;
    the output projection then uses the full Wo (no reduction needed).
  - Epilogue (bias via K=1 ones-row matmul + leaky relu on DVE); output
    is the core's 256-row slice; the host concatenates the 8 slices.
"""

import numpy as np

import concourse.bass as bass
import concourse.mybir as mybir
import concourse.tile as tile
from concourse import bacc
from concourse import bass_utils
from concourse.masks import make_identity

NC_CORES = 8
S1 = 2048
S2 = 2048
D1 = 1024
D2 = 1024
H, DK, DV = 16, 64, 64
HPC = H // NC_CORES          # heads per core = 2
EPC = HPC * DK               # projection dims per core = 128
SPC = S1 // NC_CORES         # output rows per core = 256
P = 128
F32 = mybir.dt.float32
F32R = mybir.dt.float32r
ACT_EXP = mybir.ActivationFunctionType.Exp
ACT_LRELU = mybir.ActivationFunctionType.Lrelu
ACT_LN = mybir.ActivationFunctionType.Ln
MAX = mybir.AluOpType.max

NEG_SLOPE = 0.01
SCALE = 1.0 / np.sqrt(np.float32(DK))   # 0.125

S2_T = S2 // P               # 16 key tiles
KD1 = D1 // P                # 8 contraction tiles for projections
KDV = (H * DV) // P          # 8 contraction tiles for out projection


BF16 = mybir.dt.bfloat16
MM_DTYPE = "bf16"  # "bf16" | "f32r" | "f32" for matmul operand tiles


def build(mm_dtype: str = MM_DTYPE, single_core: bool = False):
    """single_core=True swaps the AllToAll for a local DMA copy (its exact
    1-core semantics) so the kernel can run in TimelineSim for perf
    estimation."""
    mmdt = {"bf16": BF16, "f32r": F32R, "f32": F32}[mm_dtype]
    nc = bacc.Bacc("TRN2", target_bir_lowering=False, debug=False,
                   num_devices=1 if single_core else NC_CORES)

    x1T = nc.dram_tensor("x1T", [D1, S1], mmdt, kind="ExternalInput")
    x2T = nc.dram_tensor("x2T", [D2, S2], mmdt, kind="ExternalInput")
    wqT = nc.dram_tensor("wqT", [D1, EPC], mmdt, kind="ExternalInput")
    wkT = nc.dram_tensor("wkT", [D2, EPC], mmdt, kind="ExternalInput")
    wvT = nc.dram_tensor("wvT", [D2, EPC], mmdt, kind="ExternalInput")
    woT = nc.dram_tensor("woT", [H * DV, D1], mmdt, kind="ExternalInput")
    bo_bc = nc.dram_tensor("bo_bc", [1, D1], F32, kind="ExternalInput")
    out = nc.dram_tensor("out", [SPC, D1], F32, kind="ExternalOutput")
    # exchange payload in the matmul dtype (bf16 halves the collective)
    import os as _osA
    a2a_dt = {"bf16": BF16, "f32": F32}[_osA.environ.get("A2A_DT", "bf16")] \
        if mmdt == BF16 else F32
    a2a_shared = _osA.environ.get("A2A_SHARED", "0") == "1"
    a2a_in = [nc.dram_tensor(f"a2a_in{h}", [NC_CORES * DV, SPC], a2a_dt,
                             kind="Internal") for h in range(HPC)]
    a2a_out = [nc.dram_tensor(f"a2a_out{h}", [NC_CORES * DV, SPC], a2a_dt,
                              kind="Internal",
                              addr_space="Shared" if a2a_shared else "Local")
              for h in range(HPC)]

    import os as _os0
    _ptb = int(_os0.environ.get("PTB", "5"))
    _xtb = int(_os0.environ.get("XTB", "10"))
    _psb = int(_os0.environ.get("PSB", "2"))
    _pob = int(_os0.environ.get("POB", "4"))
    _dgr = int(_os0.environ.get("DGR", "1"))

    with tile.TileContext(nc) as tc:
        with (
            tc.tile_pool(name="const", bufs=1) as cpool,
            tc.tile_pool(name="res", bufs=1) as rpool,
            tc.tile_pool(name="xin", bufs=_xtb) as xpool,
            tc.tile_pool(name="lhs", bufs=2) as lpool,
            tc.tile_pool(name="pt", bufs=_ptb) as ptpool,
            tc.tile_pool(name="ytmp", bufs=2) as ypool,
            tc.tile_pool(name="norm", bufs=2) as npool,
            tc.tile_pool(name="ps", bufs=_psb, space="PSUM") as pspool,
            tc.tile_pool(name="po", bufs=_pob, space="PSUM") as popool,
        ):
            # ---------------- constants (wo/bias deferred) ----------------
            ident = cpool.tile([P, P], F32 if mmdt == F32R else mmdt)
            make_identity(nc, ident[:])
            wq_sb = cpool.tile([P, KD1, EPC], mmdt)
            wk_sb = cpool.tile([P, KD1, EPC], mmdt)
            wv_sb = cpool.tile([P, KD1, EPC], mmdt)
            nc.sync.dma_start(wk_sb[:], wkT.rearrange("(o p) m -> p o m", p=P))
            nc.sync.dma_start(wv_sb[:], wvT.rearrange("(o p) m -> p o m", p=P))
            nc.sync.dma_start(wq_sb[:], wqT.rearrange("(o p) m -> p o m", p=P))
            wo_sb = cpool.tile([P, KDV, D1], mmdt)
            bo_r = cpool.tile([1, D1], F32R)
            ones_r = cpool.tile([1, P], F32R)
            nc.vector.memset(ones_r[:].bitcast(F32), 1.0)
            # all-ones row used as K=1 matmul lhsT to broadcast the softmax
            # denominator reciprocal across the DV output partitions
            ones_t = cpool.tile([1, DV], F32R)
            nc.vector.memset(ones_t[:].bitcast(F32), 1.0)

            # ---------------- residents ----------------
            vT = rpool.tile([P, S2], mmdt, name="vT")
            # per-head q^T/k^T, kept at their natural partition range
            # (h=0 -> rows 0:64, h=1 -> rows 64:128) with the other half
            # zeroed: partition-aligned DVE copies straight from the
            # projection PSUM, and the K=128 matmul sums the zeros away.
            qTh = [rpool.tile([P, S1], mmdt, name=f"qT{h}") for h in range(HPC)]
            kTh = [rpool.tile([P, S2], mmdt, name=f"kT{h}") for h in range(HPC)]
            # V natural + ones column, per key tile: [j, (v_h0|1|v_h1|1)]
            vP = rpool.tile([P, S2_T, 2 * (DV + 1)], mmdt)
            oTh = [rpool.tile([DV, S1], a2a_dt, name=f"oT{h}")
                   for h in range(HPC)]

            def msview(ap):
                return ap.bitcast(F32) if mmdt == F32R else ap

            for h in range(HPC):
                pad = slice(DK, P) if h == 0 else slice(0, DK)
                nc.vector.memset(msview(qTh[h][pad, :]), 0.0)
                nc.vector.memset(msview(kTh[h][pad, :]), 0.0)
            nc.vector.memset(msview(vP[:, :, DV:DV + 1]), 1.0)
            nc.vector.memset(msview(vP[:, :, 2 * DV + 1:2 * DV + 2]), 1.0)

            # ---------------- projections ----------------
            # K and V share one pass over x2T (each x2 tile DMA'd once).
            # x2 loads issue from gpsimd/SWDGE, x1 loads from SP/HWDGE to
            # spread DMA-issue cost across sequencers.
            x2v = x2T.rearrange("(o p) i -> p o i", p=P)
            x1v = x1T.rearrange("(o p) i -> p o i", p=P)

            def kv_proj(gp):
                gsl = slice(gp * 1024, (gp + 1) * 1024)
                pk = pspool.tile([P, 1024], F32, tag="ps", name=f"pk{gp}")
                pv = pspool.tile([P, 1024], F32, tag="ps", name=f"pv{gp}")
                for dg in range(KD1 // _dgr):
                    xt = xpool.tile([P, _dgr, 1024], mmdt, tag="xt",
                                    name=f"xt2_{gp}_{dg}")
                    # scalar/ACT + sync rings are only free of other
                    # work for gp0; round-robin 3 rings there
                    if gp > 0:
                        eng = nc.gpsimd
                    else:
                        eng = (nc.gpsimd, nc.scalar, nc.sync)[dg % 3]
                    eng.dma_start(
                        xt[:], x2v[:, _dgr * dg:_dgr * (dg + 1), gsl])
                    for dd in range(_dgr):
                        d = _dgr * dg + dd
                        for sg in range(2):
                            nc.tensor.matmul(
                                pk[:, sg * 512:(sg + 1) * 512],
                                wk_sb[:, d, :],
                                xt[:, dd, sg * 512:(sg + 1) * 512],
                                start=(d == 0), stop=(d == KD1 - 1))
                            nc.tensor.matmul(
                                pv[:, sg * 512:(sg + 1) * 512],
                                wv_sb[:, d, :],
                                xt[:, dd, sg * 512:(sg + 1) * 512],
                                start=(d == 0), stop=(d == KD1 - 1))
                nc.vector.tensor_copy(vT[:, gsl], pv[:])
                for h in range(HPC):
                    dat = slice(h * DK, (h + 1) * DK)
                    nc.vector.tensor_copy(kTh[h][dat, gsl], pk[dat, :])

            def q_proj(gp):
                gsl = slice(gp * 1024, (gp + 1) * 1024)
                pq = pspool.tile([P, 1024], F32, tag="ps", name=f"pq{gp}")
                for dg in range(KD1 // _dgr):
                    xt = xpool.tile([P, _dgr, 1024], mmdt, tag="xt",
                                    name=f"xt1_{gp}_{dg}")
                    nc.sync.dma_start(
                        xt[:], x1v[:, _dgr * dg:_dgr * (dg + 1), gsl])
                    for dd in range(_dgr):
                        d = _dgr * dg + dd
                        for sg in range(2):
                            nc.tensor.matmul(
                                pq[:, sg * 512:(sg + 1) * 512],
                                wq_sb[:, d, :],
                                xt[:, dd, sg * 512:(sg + 1) * 512],
                                start=(d == 0), stop=(d == KD1 - 1))
                for h in range(HPC):
                    dat = slice(h * DK, (h + 1) * DK)
                    nc.vector.tensor_copy(qTh[h][dat, gsl], pq[dat, :])

            def v_transpose(half):
                ptr = pspool.tile([P, 1024], mmdt if mmdt == BF16 else F32,
                                  tag="ps", name=f"ptr{half}")
                for k in range(8):
                    t = 8 * half + k
                    nc.tensor.transpose(
                        ptr[:, k * P:(k + 1) * P],
                        vT[:, t * P:(t + 1) * P].bitcast(F32)
                        if mmdt == F32R else vT[:, t * P:(t + 1) * P],
                        ident[:])
                for k in range(8):
                    t = 8 * half + k
                    nc.vector.tensor_copy(
                        vP[:, t, 0:DV], ptr[:, k * P:k * P + DV])
                    nc.vector.tensor_copy(
                        vP[:, t, DV + 1:2 * DV + 1],
                        ptr[:, k * P + DV:(k + 1) * P])

            # ---------------- attention ----------------
            po_tiles = {}

            def attn(h, ih, t0, t1):
                if (h, ih) not in po_tiles:
                    po_tiles[(h, ih)] = [
                        popool.tile([DV + 1, 512], F32, tag="po",
                                    name=f"po_{h}_{ih}_{gg}")
                        for gg in range(2)]
                po = po_tiles[(h, ih)]
                for t in range(t0, t1):
                    sps = pspool.tile([P, 1024], F32, tag="ps",
                                      name=f"sps_{h}_{ih}_{t}")
                    for sg in range(2):
                        i0 = ih * 1024 + sg * 512
                        nc.tensor.matmul(
                            sps[:, sg * 512:(sg + 1) * 512],
                            kTh[h][:, t * P:(t + 1) * P],
                            qTh[h][:, i0:i0 + 512],
                            start=True, stop=True)
                    ptt = ptpool.tile([P, 1024], mmdt, tag="ptt",
                                      name=f"ptt_{h}_{ih}_{t}")
                    nc.scalar.activation(ptt[:], sps[:], ACT_EXP,
                                         scale=float(SCALE))
                    for sg in range(2):
                        nc.tensor.matmul(
                            po[sg][:],
                            vP[:, t, h * (DV + 1):(h + 1) * (DV + 1)],
                            ptt[:, sg * 512:(sg + 1) * 512],
                            start=(t == 0), stop=(t == S2_T - 1))

            sr_tiles = {}

            def fin_recip(h, ih, use_act=False):
                # deferred reciprocal of the denominator row; DVE while the
                # exp table is still in use, ACT 1/Z = Exp(-Ln(Z)) for the
                # final one (all score exps done -> one table switch)
                po = po_tiles[(h, ih)]
                srs = []
                for gg in range(2):
                    g = ih * 2 + gg
                    sf = npool.tile([1, 512], F32, tag="sf",
                                    name=f"sf_{h}_{g}")
                    sr = npool.tile([1, 512], F32R, tag="sr",
                                    name=f"sr_{h}_{g}")
                    if use_act:
                        nc.scalar.activation(sf[:], po[gg][DV:DV + 1, :],
                                             ACT_LN)
                        nc.scalar.activation(sr[:], sf[:], ACT_EXP,
                                             scale=-1.0)
                    else:
                        nc.vector.reciprocal(sf[:], po[gg][DV:DV + 1, :])
                        # the copy rounds to f32r (for the bc matmul)
                        nc.vector.tensor_copy(sr[:], sf[:])
                    srs.append(sr)
                sr_tiles[(h, ih)] = srs

            def fin_apply(h, ih):
                # broadcast 1/Z over DV partitions (K=1 matmul), normalize
                # out of PSUM into the bf16 exchange buffer
                po = po_tiles.pop((h, ih))
                srs = sr_tiles.pop((h, ih))
                for gg in range(2):
                    g = ih * 2 + gg
                    gs = slice(g * 512, (g + 1) * 512)
                    bc = pspool.tile([DV, 512], F32, tag="ps",
                                     name=f"bc_{h}_{g}")
                    nc.tensor.matmul(
                        bc[:],
                        ones_t[:],
                        srs[gg][:],
                        start=True, stop=True)
                    nc.vector.tensor_copy(oTh[h][:, gs], po[gg][0:DV, :])
                    nc.vector.tensor_mul(
                        oTh[h][:, gs], oTh[h][:, gs], bc[:])

            def scatter_half(h, half):
                # peers 0-3 correspond to query cols 0:1024 (ih=0), peers
                # 4-7 to cols 1024:2048 -- issue each half right after the
                # matching fin_apply so the last pre-collective DMA is small
                jsl = slice(half * 4, (half + 1) * 4)
                nc.sync.dma_start(
                    a2a_in[h].rearrange("(j p) i -> p j i", p=DV)[:, jsl, :],
                    oTh[h][:].rearrange("p (j i) -> p j i", j=NC_CORES)
                             [:, jsl, :])

            def exchange(h):
                if single_core:
                    nc.sync.dma_start(a2a_out[h][:], a2a_in[h][:])
                else:
                    nc.gpsimd.collective_compute(
                        "AllToAll", mybir.AluOpType.bypass,
                        replica_groups=[list(range(NC_CORES))],
                        ins=[a2a_in[h][:].opt()],
                        outs=[a2a_out[h][:].opt()],
                    )

            # out-proj lhsT split per exchange: ltA = [even heads | 0],
            # ltB = [0 | odd heads] so the even-head half of the projection
            # can run (K=128, full-rate) while the second AllToAll flies.
            ltA = [lpool.tile([P, KDV, P], mmdt, tag="ltA", name=f"ltA{it}")
                   for it in range(SPC // P)]
            ltB = [lpool.tile([P, KDV, P], mmdt, tag="ltB", name=f"ltB{it}")
                   for it in range(SPC // P)]
            for it in range(SPC // P):
                nc.vector.memset(msview(ltA[it][DV:P, :, :]), 0.0)
                nc.vector.memset(msview(ltB[it][0:DV, :, :]), 0.0)

            def lt_load(h):
                # prefetch this head's half of the out-proj lhsT row blocks
                lt = ltA if h == 0 else ltB
                for it in range(SPC // P):
                    nc.gpsimd.dma_start(
                        lt[it][h * DV:(h + 1) * DV, :, :],
                        a2a_out[h].rearrange("(k p) i -> p k i", p=DV)
                                  [:, :, it * P:(it + 1) * P])

            pys = []

            def outproj_phaseA():
                # even heads (zero-padded to K=128): runs while the second
                # AllToAll is in flight, keeps the PE p-state warm
                for it in range(SPC // P):
                    py = pspool.tile([P, D1], F32, tag="ps")
                    pys.append(py)
                    for k in range(KDV):
                        for ng in range(2):
                            nc.tensor.matmul(
                                py[:, ng * 512:(ng + 1) * 512],
                                ltA[it][:, k, :],
                                wo_sb[:, k, ng * 512:(ng + 1) * 512],
                                start=(k == 0), stop=False,
                                skip_group_check=True)

            # ---------------- emission ----------------
            import os as _os
            _order = _os.environ.get("KERNEL_EMIT_ORDER", "ilv")

            def load_wo():
                nc.sync.dma_start(wo_sb[:],
                                  woT.rearrange("(o p) m -> p o m", p=P))
                nc.gpsimd.dma_start(bo_r[:], bo_bc[0:1, :])

            if _order == "ilv":
                kv_proj(0)
                q_proj(0)
                v_transpose(0)
                attn(0, 0, 0, 8)
                kv_proj(1)
                q_proj(1)
                v_transpose(1)
                attn(0, 0, 8, S2_T)
                load_wo()
                attn(0, 1, 0, 1)
                fin_recip(0, 0)
                attn(0, 1, 1, S2_T)
                fin_apply(0, 0)
                scatter_half(0, 0)
                attn(1, 0, 0, 1)
                fin_recip(0, 1)
                attn(1, 0, 1, S2_T)
                fin_apply(0, 1)
                scatter_half(0, 1)
                exchange(0)
                lt_load(0)
                attn(1, 1, 0, 1)
                fin_recip(1, 0)
                attn(1, 1, 1, S2_T)
                fin_apply(1, 0)
                scatter_half(1, 0)
                fin_recip(1, 1, use_act=True)
                fin_apply(1, 1)
                scatter_half(1, 1)
                outproj_phaseA()
                exchange(1)
                lt_load(1)
            else:  # "seq"
                kv_proj(0)
                v_transpose(0)
                q_proj(0)
                kv_proj(1)
                v_transpose(1)
                q_proj(1)
                load_wo()
                for _h in range(HPC):
                    for _ih in range(2):
                        attn(_h, _ih, 0, S2_T)
                        fin_recip(_h, _ih)
                        fin_apply(_h, _ih)
                    scatter_half(_h, 0)
                    scatter_half(_h, 1)
                    if _h == HPC - 1:
                        exchange(_h)
                        outproj_phaseA()
                    else:
                        exchange(_h)
                    lt_load(_h)

            # ---------------- output projection + epilogue ----------------
            # (phase A was emitted before the second AllToAll launch)
            for it in range(SPC // P):
                py = pys[it]
                for k in range(KDV):
                    for ng in range(2):
                        nc.tensor.matmul(
                            py[:, ng * 512:(ng + 1) * 512],
                            ltB[it][:, k, :],
                            wo_sb[:, k, ng * 512:(ng + 1) * 512],
                            start=False, stop=False,
                            skip_group_check=True)
                # bias folded into the same PSUM accumulation (K=1 ones row)
                for ng in range(2):
                    nc.tensor.matmul(
                        py[:, ng * 512:(ng + 1) * 512],
                        ones_r[:],
                        bo_r[:, ng * 512:(ng + 1) * 512],
                        start=False, stop=True, skip_group_check=True)
                ysb = ypool.tile([P, D1], F32, tag="ysb")
                yml = ypool.tile([P, D1], F32, tag="yml")
                nc.vector.tensor_scalar_mul(yml[:], py[:], NEG_SLOPE)
                nc.vector.tensor_tensor(ysb[:], py[:], yml[:], MAX)
                nc.sync.dma_start(out[it * P:(it + 1) * P, :], ysb[:])

    nc.compile()
    return nc


_NC_CACHE = {}


def _get_nc():
    if "nc" not in _NC_CACHE:
        _NC_CACHE["nc"] = build()
    return _NC_CACHE["nc"]


def make_in_maps(x1, x2, Wq, Wk, Wv, Wo, bo, mm_dtype: str = MM_DTYPE):
    import ml_dtypes
    cast = (lambda a: a.astype(ml_dtypes.bfloat16)) if mm_dtype == "bf16" \
        else (lambda a: a)
    x1 = np.asarray(x1, dtype=np.float32)
    x2 = np.asarray(x2, dtype=np.float32)
    Wq = np.asarray(Wq, dtype=np.float32)
    Wk = np.asarray(Wk, dtype=np.float32)
    Wv = np.asarray(Wv, dtype=np.float32)
    Wo = np.asarray(Wo, dtype=np.float32)
    bo = np.asarray(bo, dtype=np.float32)
    x1T = cast(np.ascontiguousarray(x1.T))
    x2T = cast(np.ascontiguousarray(x2.T))
    woT = cast(np.ascontiguousarray(Wo.T))
    bo_bc = np.ascontiguousarray(bo.reshape(1, D1))
    in_maps = []
    for c in range(NC_CORES):
        sl = slice(EPC * c, EPC * (c + 1))
        in_maps.append({
            "x1T": x1T,
            "x2T": x2T,
            "wqT": cast(np.ascontiguousarray(Wq[sl, :].T)),
            "wkT": cast(np.ascontiguousarray(Wk[sl, :].T)),
            "wvT": cast(np.ascontiguousarray(Wv[sl, :].T)),
            "woT": woT,
            "bo_bc": bo_bc,
        })
    return in_maps


def _install_profile_shim():
    """The image's antenv lacks axon_hooks; shim it so trace=True can pull
    NTFF profiles (exec_time_ns) through the axon tunnel."""
    import sys as _sys
    import types as _types
    try:
        from antenv.axon_hooks import get_axon_ntff_profile_hook  # noqa: F401
        return
    except ImportError:
        pass
    try:
        from trn_agent_boot.trn_boot import _ntff_profile_via_ctypes
        hook = _ntff_profile_via_ctypes("/opt/axon/libaxon_pjrt.so")
        mod = _types.ModuleType("antenv.axon_hooks")
        mod.get_axon_ntff_profile_hook = lambda: hook
        mod.set_axon_ntff_profile_hook = lambda h: None
        _sys.modules["antenv.axon_hooks"] = mod
        bass_utils.upload_artifacts = lambda tmpdir: tmpdir
    except Exception:
        pass


def run(inputs, trace=False):
    if trace:
        _install_profile_shim()
    nc = _get_nc()
    in_maps = make_in_maps(**inputs)
    res = bass_utils.run_bass_kernel_spmd(
        nc, in_maps, core_ids=list(range(NC_CORES)), trace=trace)
    full = np.concatenate(
        [res.results[c]["out"] for c in range(NC_CORES)], axis=0)
    return full, res


def kernel(**inputs):
    full, _ = run(inputs, trace=False)
    return full
